# revision 1
# baseline (speedup 1.0000x reference)
"""BFPLinear Trainium2 kernel.

Computes: out = bfp_quantize(x) @ bfp_quantize(w).T + 2*bias
where bfp_quantize is 8-bit block-floating-point with shared-exponent
groups of 32 along the last (in_features) dim.

Sharding across 8 NeuronCores: 2 batch-groups x 4 column-groups.
Each core gets x[4096, 4096], w[1024, 4096], bias2[1024] and produces
out[4096, 1024].

On-core pipeline per 128-row strip (processed in 2048-col chunks):
  1.  DMA load (fp32, natural layout)
  2.  DVE grouped abs-max reduce (groups of 32 along free dim)
  3.  exponent bit-tricks -> rstep = 2^(6-e), step = 2^(e-6)
  4.  q = x * rstep  (TT mul, broadcast per group)  [DVE/GPSIMD split]
  5.  round+clip via magic-number dual-op tensor_scalars (DVE)
  6.  xq_bf16 = qc * step (exact in bf16)           [GPSIMD]
  7.  xbar DMA transpose -> [K partitions, rows] bf16 tiles
  8.  PE matmul accumulate over K into PSUM, bias add on evict
"""

import os
import numpy as np

import concourse.bass as bass
import concourse.bacc as bacc
import concourse.tile as tile
import concourse.mybir as mybir
from concourse.bass_utils import run_bass_kernel_spmd

F32 = mybir.dt.float32
BF16 = mybir.dt.bfloat16
U32 = mybir.dt.uint32
ALU = mybir.AluOpType
AX = mybir.AxisListType

# Full problem
B_FULL, IN_FULL, OUT_FULL = 8192, 4096, 4096
NBATCH, NCOL = 2, 4  # 2 batch-groups x 4 col-groups = 8 cores
SM_FULL = B_FULL // NBATCH    # 2048 rows of x per core
SN_FULL = OUT_FULL // NCOL    # 2048 output cols per core

MAGIC = 12582912.0            # 1.5 * 2^23: fp32 round-to-int magic
GROUP = 32
QMAX = 127.0


def _bcast_group(t_ap, g, e=GROUP):
    """View a [128, g] tile as [128, g, e] with the inner dim broadcast."""
    return bass.AP(
        tensor=t_ap.tensor,
        offset=t_ap.offset,
        ap=[t_ap.ap[0], t_ap.ap[1], [0, e]],
    )


def build_bass(SM=SM_FULL, SN=SN_FULL, K=IN_FULL, CH=2048):
    """Build the per-core Bass program.

    SM: rows of x shard; SN: rows of w shard (= output cols);
    K: contraction dim; CH: quantization chunk size (divides K,
    multiple of 128).
    """
    assert K % CH == 0 and CH % 128 == 0 and CH % GROUP == 0
    NKT = K // 128          # k-tiles
    CHT = CH // 128         # k-tiles per chunk
    G = CH // GROUP         # groups per chunk
    NCH = K // CH           # chunks per strip
    MS = SM // 128          # m-strips
    NS = SN // 128          # n-strips
    NSL = (SN + 511) // 512  # 512-wide n slices per psum

    nc = bacc.Bacc("TRN2", target_bir_lowering=False)

    x = nc.dram_tensor("x", [SM, K], F32, kind="ExternalInput")
    w = nc.dram_tensor("w", [SN, K], F32, kind="ExternalInput")
    b2 = nc.dram_tensor("b2", [SN], F32, kind="ExternalInput")
    o = nc.dram_tensor("o", [SM, SN], F32, kind="ExternalOutput")

    with tile.TileContext(nc) as tc:
        with (
            tc.tile_pool(name="res", bufs=1) as res_p,
            tc.tile_pool(name="nat", bufs=4) as nat_p,
            tc.tile_pool(name="qp", bufs=4) as q_p,
            tc.tile_pool(name="qb", bufs=4) as qb_p,
            tc.tile_pool(name="tiny", bufs=6) as tiny_p,
            tc.tile_pool(name="xqt", bufs=4) as xqt_p,
            tc.tile_pool(name="outp", bufs=2) as out_p,
            tc.tile_pool(name="psum", bufs=4, space="PSUM") as psum_p,
        ):
            wqT = res_p.tile([128, NKT, SN], BF16)
            bias2 = res_p.tile([128, SN], F32)
            const82 = res_p.tile([128, G], U32)

            # bias2 broadcast along partitions from the 1-D dram vector
            nc.sync.dma_start(
                out=bias2,
                in_=bass.AP(tensor=b2, offset=0, ap=[[0, 128], [1, SN]]),
            )
            nc.vector.memset(const82, 0x82000000)

            qidx = [0]

            def quant_chunk(src_slice, dst3d, tt2_dve=False):
                """Quantize one [128, CH] fp32 chunk and write its
                transposed bf16 k-tiles into dst3d ([128, CHT, 128])."""
                i = qidx[0]
                qidx[0] += 1
                nat = nat_p.tile([128, CH], F32, tag="nat")
                nc.sync.dma_start(out=nat, in_=src_slice)
                nat3 = nat[:].rearrange("p (g e) -> p g e", e=GROUP)

                gmax = tiny_p.tile([128, G], F32, tag="gmax")
                nc.vector.tensor_reduce(
                    out=gmax[:], in_=nat3, axis=AX.X, op=ALU.max,
                    apply_absolute_value=True,
                )
                # keep only the exponent: gexp = 2^floor(log2(gmax))
                nc.vector.tensor_scalar(
                    out=gmax[:].bitcast(U32), in0=gmax[:].bitcast(U32),
                    scalar1=0x7F800000, scalar2=None, op0=ALU.bitwise_and,
                )
                # rstep = 2^(6-e): bits = 0x82000000 - gexp_bits
                rstep = tiny_p.tile([128, G], F32, tag="rstep")
                nc.vector.tensor_tensor(
                    out=rstep[:].bitcast(U32), in0=const82[:],
                    in1=gmax[:].bitcast(U32), op=ALU.subtract,
                )
                # step = 2^(e-6) = gexp * 2^-6 (exact fp mul)
                step = tiny_p.tile([128, G], F32, tag="step")
                nc.vector.tensor_scalar(
                    out=step[:], in0=gmax[:], scalar1=0.015625,
                    scalar2=None, op0=ALU.mult,
                )

                q = q_p.tile([128, CH], F32, tag="q")
                q3 = q[:].rearrange("p (g e) -> p g e", e=GROUP)
                eng1 = nc.vector if (i % 2 == 0) else nc.gpsimd
                eng1.tensor_tensor(
                    out=q3, in0=nat3, in1=_bcast_group(rstep[:], G), op=ALU.mult
                )
                # round to nearest-even + clip to [-127, 127], fused via the
                # fp32 magic constant: qc = max(min(q + M, M+127), M-127) - M
                nc.vector.tensor_scalar(
                    out=q[:], in0=q[:], scalar1=MAGIC, scalar2=MAGIC + QMAX,
                    op0=ALU.add, op1=ALU.min,
                )
                nc.vector.tensor_scalar(
                    out=q[:], in0=q[:], scalar1=MAGIC - QMAX, scalar2=MAGIC,
                    op0=ALU.max, op1=ALU.subtract,
                )
                # xq = qc * step, exact in bf16
                qb = qb_p.tile([128, CH], BF16, tag="qb")
                qb3 = qb[:].rearrange("p (g e) -> p g e", e=GROUP)
                eng2 = nc.vector if tt2_dve else nc.gpsimd
                eng2.tensor_tensor(
                    out=qb3, in0=q3, in1=_bcast_group(step[:], G), op=ALU.mult
                )
                # blocked xbar transpose: [128, CH] -> [128, CHT, 128]
                nc.scalar.dma_start_transpose(out=dst3d, in_=qb[:])

            # ---- W phase: quantize+transpose all of w into resident wqT
            for s in range(NS):
                for h in range(NCH):
                    quant_chunk(
                        w[s * 128:(s + 1) * 128, h * CH:(h + 1) * CH],
                        wqT[:, h * CHT:(h + 1) * CHT, s * 128:(s + 1) * 128],
                        tt2_dve=(s % 4 == 3),
                    )

            # ---- X phase: per m-strip quantize, transpose, matmul, evict
            for m in range(MS):
                xqt = xqt_p.tile([128, NKT, 128], BF16, tag="xqt")
                for h in range(NCH):
                    quant_chunk(
                        x[m * 128:(m + 1) * 128, h * CH:(h + 1) * CH],
                        xqt[:, h * CHT:(h + 1) * CHT, :],
                    )
                psum = psum_p.tile([128, SN], F32, tag="psum")
                for kt in range(NKT):
                    for nj in range(NSL):
                        n0 = nj * 512
                        n1 = min(SN, n0 + 512)
                        nc.tensor.matmul(
                            psum[:, n0:n1],
                            xqt[:, kt, :],
                            wqT[:, kt, n0:n1],
                            start=(kt == 0),
                            stop=(kt == NKT - 1),
                        )
                outt = out_p.tile([128, SN], F32, tag="outt")
                nc.vector.tensor_tensor(
                    out=outt[:], in0=psum[:], in1=bias2[:], op=ALU.add
                )
                nc.sync.dma_start(
                    out=o[m * 128:(m + 1) * 128, :], in_=outt[:]
                )

    nc.compile()
    return nc


_NC_CACHE = {}


def _get_nc(key=("full",)):
    if key not in _NC_CACHE:
        if key == ("full",):
            _NC_CACHE[key] = build_bass()
        else:
            _NC_CACHE[key] = build_bass(*key)
    return _NC_CACHE[key]


def kernel(input, weight, bias):
    input = np.ascontiguousarray(input, dtype=np.float32)
    weight = np.ascontiguousarray(weight, dtype=np.float32)
    bias = np.ascontiguousarray(bias, dtype=np.float32)

    nc = _get_nc()
    b2_full = bias * np.float32(2.0)

    in_maps = []
    for c in range(8):
        bi, ni = divmod(c, NCOL)
        in_maps.append({
            "x": input[bi * SM_FULL:(bi + 1) * SM_FULL, :],
            "w": weight[ni * SN_FULL:(ni + 1) * SN_FULL, :],
            "b2": b2_full[ni * SN_FULL:(ni + 1) * SN_FULL],
        })

    trace = bool(int(os.environ.get("BFP_TRACE", "0")))
    res = run_bass_kernel_spmd(
        nc, in_maps, core_ids=list(range(8)), trace=trace,
    )
    kernel.last_results = res

    out = np.empty((B_FULL, OUT_FULL), dtype=np.float32)
    for c in range(8):
        bi, ni = divmod(c, NCOL)
        out[bi * SM_FULL:(bi + 1) * SM_FULL,
            ni * SN_FULL:(ni + 1) * SN_FULL] = res.results[c]["o"]
    return out


def build_noop(SM=SM_FULL, SN=SN_FULL, K=IN_FULL):
    """Same external tensors as build_bass, near-zero device work.
    Used to subtract the (large) axon per-execute overhead, which scales
    with I/O bytes, from the real kernel's measured time."""
    nc = bacc.Bacc("TRN2", target_bir_lowering=False)
    x = nc.dram_tensor("x", [SM, K], F32, kind="ExternalInput")
    w = nc.dram_tensor("w", [SN, K], F32, kind="ExternalInput")
    b2 = nc.dram_tensor("b2", [SN], F32, kind="ExternalInput")
    o = nc.dram_tensor("o", [SM, SN], F32, kind="ExternalOutput")
    with tile.TileContext(nc) as tc:
        with tc.tile_pool(name="p", bufs=1) as p:
            t = p.tile([128, 128], F32)
            nc.sync.dma_start(out=t, in_=x[:128, :128])
            nc.sync.dma_start(out=o[:128, :128], in_=t)
            t2 = p.tile([128, 128], F32)
            nc.sync.dma_start(out=t2, in_=w[:128, :128])
            nc.sync.dma_start(out=o[:128, 128:256], in_=t2)
            t3 = p.tile([1, SN], F32)
            nc.sync.dma_start(out=t3, in_=bass.AP(tensor=b2, offset=0, ap=[[0, 1], [1, SN]]))
            nc.sync.dma_start(out=o[128:129, :], in_=t3)
    nc.compile()
    return nc


def _make_runner(nc):
    import jax
    from jax.sharding import Mesh, PartitionSpec
    from jax.experimental.shard_map import shard_map
    from concourse import bass2jax as b2j
    import concourse.mybir as mybir_

    b2j.install_neuronx_cc_hook()
    partition_name = (
        nc.partition_id_tensor.name if nc.partition_id_tensor else None
    )
    in_names, out_names, out_avals = [], [], []
    for alloc in nc.m.functions[0].allocations:
        if not isinstance(alloc, mybir_.MemoryLocationSet):
            continue
        name = alloc.memorylocations[0].name
        if alloc.kind == "ExternalInput":
            if name != partition_name:
                in_names.append(name)
        elif alloc.kind == "ExternalOutput":
            out_names.append(name)
            out_avals.append(jax.core.ShapedArray(
                tuple(alloc.tensor_shape), mybir_.dt.np(alloc.dtype)))
    n_params = len(in_names)
    all_names = list(in_names) + list(out_names)
    if partition_name is not None:
        all_names.append(partition_name)

    def _body(*args):
        operands = list(args)
        if partition_name is not None:
            operands.append(b2j.partition_id_tensor())
        return tuple(b2j._bass_exec_p.bind(
            *operands,
            out_avals=tuple(out_avals),
            in_names=tuple(all_names),
            out_names=tuple(out_names),
            lowering_input_output_aliases=(),
            sim_require_finite=True,
            sim_require_nnan=True,
            nc=nc,
        ))

    devices = jax.devices()[:8]
    mesh = Mesh(np.asarray(devices), ("core",))
    n_outs = len(out_names)
    fn = jax.jit(
        shard_map(
            _body, mesh=mesh,
            in_specs=(PartitionSpec("core"),) * (n_params + n_outs),
            out_specs=(PartitionSpec("core"),) * n_outs,
            check_rep=False,
        ),
        keep_unused=True,
    )
    return fn, in_names, out_avals, mesh


def bench(ins, iters=6):
    """Estimate per-execution device time of the 8-core kernel.

    The axon PJRT path has a large fixed+per-byte round-trip overhead,
    so we time the real kernel and a no-op NEFF with identical external
    I/O, and report the difference."""
    import time
    import jax
    from jax.sharding import PartitionSpec, NamedSharding

    input_ = np.ascontiguousarray(ins["input"], dtype=np.float32)
    weight = np.ascontiguousarray(ins["weight"], dtype=np.float32)
    b2_full = np.ascontiguousarray(ins["bias"], dtype=np.float32) * np.float32(2.0)

    shard_arrays = {
        "x": np.concatenate([input_[(c // NCOL) * SM_FULL:(c // NCOL + 1) * SM_FULL, :] for c in range(8)], axis=0),
        "w": np.concatenate([weight[(c % NCOL) * SN_FULL:(c % NCOL + 1) * SN_FULL, :] for c in range(8)], axis=0),
        "b2": np.concatenate([b2_full[(c % NCOL) * SN_FULL:(c % NCOL + 1) * SN_FULL] for c in range(8)], axis=0),
    }

    results = {}
    for tag, nc in (("real", _get_nc()), ("noop", build_noop())):
        fn, in_names, out_avals, mesh = _make_runner(nc)
        sharding = NamedSharding(mesh, PartitionSpec("core"))
        dev_in = [jax.device_put(shard_arrays[nm], sharding) for nm in in_names]
        dev_zero = [
            jax.device_put(
                np.zeros((8 * a.shape[0], *a.shape[1:]), a.dtype), sharding)
            for a in out_avals
        ]
        out = fn(*dev_in, *dev_zero)
        jax.block_until_ready(out)
        best = float("inf")
        for _ in range(iters):
            t0 = time.perf_counter()
            out = fn(*dev_in, *dev_zero)
            jax.block_until_ready(out)
            best = min(best, time.perf_counter() - t0)
        results[tag] = best
        print("bench[%s]: %.3f ms" % (tag, best * 1e3))
    diff = results["real"] - results["noop"]
    print("bench diff (device exec estimate): %.3f ms" % (diff * 1e3))
    return max(1, int(diff * 1e9))


if __name__ == "__main__":
    import sys
    mode = sys.argv[1] if len(sys.argv) > 1 else "sim"
    if mode == "sim":
        # quick numerical validation in CoreSim on a small config
        from concourse.bass_interp import CoreSim
        SM, SN, K, CH = 256, 256, 512, 256
        nc = build_bass(SM, SN, K, CH)
        rng = np.random.default_rng(0)
        xin = rng.standard_normal((SM, K), dtype=np.float32)
        win = rng.uniform(-0.1, 0.1, (SN, K)).astype(np.float32)
        bin_ = rng.uniform(-0.1, 0.1, SN).astype(np.float32)

        sim = CoreSim(nc)
        sim.tensor("x")[:] = xin
        sim.tensor("w")[:] = win
        sim.tensor("b2")[:] = bin_ * 2.0
        sim.simulate(check_with_hw=False)
        got = np.array(sim.tensor("o"))

        def bfpq(v):
            g = v.reshape(v.shape[0], -1, GROUP).astype(np.float64)
            ma = np.abs(g).max(axis=-1, keepdims=True)
            e = np.floor(np.log2(np.where(ma > 0, ma, 1.0)))
            st = np.exp2(e - 6)
            qq = np.clip(np.round(g / st), -127, 127) * st
            return np.where(ma > 0, qq, 0.0).reshape(v.shape)

        exp = bfpq(xin) @ bfpq(win).T + 2.0 * bin_.astype(np.float64)
        err = np.abs(got.astype(np.float64) - exp)
        rel = err.max() / np.abs(exp).max()
        print("max abs err:", err.max(), "rel:", rel)
        assert rel < 1e-5, "numerical mismatch"
        print("SIM PASS")
    elif mode == "hw":
        import reference
        ins = {k: np.asarray(v) for k, v in reference.setup_inputs().items()}
        outp = kernel(**ins)
        print("out", outp.shape, outp.dtype)

